# revision 1
# baseline (speedup 1.0000x reference)
"""Cosine-similarity attention map on 8 Trainium2 NeuronCores.

out[b, i, j] = <x[b,:,i], x[b,:,j]> / (||x[b,:,i]|| * ||x[b,:,j]||)
x: [B=4, C=64, N=4096] fp32  ->  out: [B=4, N=4096, N=4096] fp32

Sharding: data-parallel over B (4 batches) x 2-way row-split of the N x N
output -> 8 cores. Each core receives the full x[b] (for the moving operand
and column norms) plus its 2048-column row slice (for the stationary
operand), normalizes columns on device (y = x * rsqrt(sum_c x^2)), and
computes its [2048, 4096] block of the Gram matrix of y with fp32r matmuls.
"""

import sys

sys.path.insert(0, "/opt/trn_rl_repo")

import numpy as np

import concourse.bass as bass
import concourse.mybir as mybir
import concourse.tile as tile
from concourse import bacc
from concourse.bass_utils import run_bass_kernel_spmd
from concourse.vector_clock import ScopedClock, VectorClock

B, C, N = 4, 64, 4096
NCORES = 8
RB = N * B // NCORES  # 2048 output rows per core
MM_N = 512  # moving free dim per matmul (one PSUM bank of fp32)
MM_M = 128  # output partitions per matmul
NJ = N // MM_N  # 8 column chunks
NT = RB // MM_M  # 16 row tiles per core

F32 = mybir.dt.float32
F32R = mybir.dt.float32r
F16 = mybir.dt.float16


class SplitDrainTileContext(tile.TileContext):
    """Stock TileContext attaches a wait for every pending DMA-queue
    semaphore to a single exit Drain; the walrus build here only allows one
    sync-wait per TPB_CTRL instruction ("Too many sync wait commands").
    Emit one drain per pending logical processor instead."""

    def _drain_and_barrier(self, tick_clock, wait_clock):
        gc = tick_clock.global_clock
        n = len(gc)
        for p in range(n):
            t = gc[p]
            if t <= 0:
                continue
            part = VectorClock([t if q == p else 0 for q in range(n)])
            d = self.nc.sync.drain()
            wait_clock.add_sem_waits(d.ins, ScopedClock({None: part}))

        self.nc.all_engine_barrier()
        assert self.sems is not None
        popped = self.nc._tile_sem_poison_stack.pop()
        assert popped is self._sem_poison
        self.nc.clear_and_free_semaphores(list(self.sems.allocated().values()))
        self.nc.all_engine_barrier()


def _build(use_split_drain=False):
    nc = bacc.Bacc("TRN2", target_bir_lowering=False)
    xf = nc.declare_dram_parameter("xf", [C, N], F32, isOutput=False)
    xr = nc.declare_dram_parameter("xr", [C, RB], F32, isOutput=False)
    out = nc.declare_dram_parameter("out", [RB, N], F32, isOutput=True)

    tc_cls = SplitDrainTileContext if use_split_drain else tile.TileContext
    with tc_cls(nc) as tc:
        with (
            tc.tile_pool(name="persist", bufs=1) as persist,
            tc.tile_pool(name="panels", bufs=4) as panels,
            tc.tile_pool(name="mpsum", bufs=2, space="PSUM") as mpsum,
            tc.tile_pool(name="npsum", bufs=4, space="PSUM") as npsum,
        ):
            # Load inputs, chunked so the norm pipeline starts ASAP.
            XF = persist.tile([C, N], F32)
            XR = persist.tile([C, RB], F32)
            for c0 in range(0, RB, 1024):
                nc.sync.dma_start(
                    out=XR[:, c0 : c0 + 1024], in_=xr[:, c0 : c0 + 1024]
                )
            for c0 in range(0, N, 1024):
                nc.sync.dma_start(
                    out=XF[:, c0 : c0 + 1024], in_=xf[:, c0 : c0 + 1024]
                )

            ones_f = persist.tile([C, 1], F32)
            nc.vector.memset(ones_f, 1.0)
            ones_c = persist.tile([C, 1], F16)  # sumsq reduction lhsT
            nc.vector.tensor_copy(ones_c, ones_f)
            ones_rf = persist.tile([1, C], F32)
            nc.vector.memset(ones_rf, 1.0)
            ones_r = persist.tile([1, C], F16)  # K=1 partition-broadcast lhsT
            nc.vector.tensor_copy(ones_r, ones_rf)

            # Normalize columns: y = x * rsqrt(sum_c x^2), in fp16, in
            # 1024-column chunks. Per chunk: square (DVE) -> sum over C via
            # ones-matmul (PE) -> approx reciprocal from PSUM (DVE) -> sqrt
            # to fp16 (ACT) -> partition-broadcast via K=1 matmul (PE) ->
            # y = x * bcast read from PSUM (DVE).
            CH = 512
            SQR16 = persist.tile([C, RB], F16)
            SQF16 = persist.tile([C, N], F16)
            RS = persist.tile([1, N], F32)
            RN16 = persist.tile([1, N], F16)
            RSr = persist.tile([1, RB], F32)
            RNr16 = persist.tile([1, RB], F16)
            YR = persist.tile([C, RB], F16)
            YF = persist.tile([C, N], F16)

            def norm_chunk(x_src, sq, rs, rn16, y, c0):
                cs = slice(c0, c0 + CH)
                nc.scalar.activation(
                    sq[:, cs], x_src[:, cs], mybir.ActivationFunctionType.Square
                )
                pps = npsum.tile([MM_M, MM_N], F32, tag="pps")
                nc.tensor.matmul(
                    pps[0:1, :], lhsT=ones_c, rhs=sq[:, cs], start=True, stop=True
                )
                nc.vector.reciprocal_approx_fast(rs[:, cs], pps[0:1, :])
                nc.scalar.activation(
                    rn16[:, cs], rs[:, cs], mybir.ActivationFunctionType.Sqrt
                )
                nc.tensor.matmul(
                    pps[0:C, :], lhsT=ones_r, rhs=rn16[:, cs], start=True, stop=True
                )
                nc.vector.tensor_mul(y[:, cs], x_src[:, cs], pps[0:C, :])

            for c0 in range(0, RB, CH):  # row slice first: gates lhsT
                norm_chunk(XR, SQR16, RSr, RNr16, YR, c0)

            # Engines run their queues in order, so emit panel 0's first
            # half right after the column chunks it needs (0..3) — its
            # copies would otherwise queue behind the whole preamble.
            def panel_half(panel, t, hh):
                ts_ = slice(t * MM_M, (t + 1) * MM_M)
                for h in (2 * hh, 2 * hh + 1):
                    ps = mpsum.tile([MM_M, 2 * MM_N], F32, tag="ps")
                    for q in range(2):
                        j = 2 * h + q
                        js = slice(j * MM_N, (j + 1) * MM_N)
                        nc.tensor.matmul(
                            ps[:, q * MM_N : (q + 1) * MM_N],
                            lhsT=YR[:, ts_],
                            rhs=YF[:, js],
                            start=True,
                            stop=True,
                        )
                    hs = slice(h * 1024, (h + 1) * 1024)
                    if h % 2 == 0:
                        nc.vector.tensor_copy(panel[:, hs], ps)
                    else:
                        nc.scalar.copy(out=panel[:, hs], in_=ps)
                nc.sync.dma_start(
                    out=out[ts_, 2048 * hh : 2048 * (hh + 1)],
                    in_=panel[:, 2048 * hh : 2048 * (hh + 1)],
                )

            for c0 in range(0, 4 * CH, CH):
                norm_chunk(XF, SQF16, RS, RN16, YF, c0)
            early = []
            for t in range(3):
                pnl = panels.tile([MM_M, N], F32, tag="panel")
                panel_half(pnl, t, 0)
                early.append(pnl)
            for c0 in range(4 * CH, N, CH):
                norm_chunk(XF, SQF16, RS, RN16, YF, c0)
            for t in range(3):
                panel_half(early[t], t, 1)

            # Gram matrix: out[i, j] = sum_c YR[c, i] * YF[c, j].
            # 4 matmuls fill a 4-bank PSUM tile; plain PSUM->SBUF copies
            # split between DVE (vector) and ACT (scalar); one contiguous
            # 2 MiB DMA per 128-row panel.
            for t in range(3, NT):
                panel = panels.tile([MM_M, N], F32)
                ts_ = slice(t * MM_M, (t + 1) * MM_M)
                for h in range(4):
                    ps = mpsum.tile([MM_M, 2 * MM_N], F32, tag="ps")
                    for q in range(2):
                        j = 2 * h + q
                        js = slice(j * MM_N, (j + 1) * MM_N)
                        qs = slice(q * MM_N, (q + 1) * MM_N)
                        nc.tensor.matmul(
                            ps[:, qs],
                            lhsT=YR[:, ts_],
                            rhs=YF[:, js],
                            start=True,
                            stop=True,
                        )
                    hs = slice(h * 1024, (h + 1) * 1024)
                    if h % 2 == 0:
                        nc.vector.tensor_copy(panel[:, hs], ps)
                    else:
                        nc.scalar.copy(out=panel[:, hs], in_=ps)
                    if h % 2 == 1:
                        nc.sync.dma_start(
                            out=out[ts_, 2048 * (h // 2) : 2048 * (h // 2 + 1)],
                            in_=panel[:, 2048 * (h // 2) : 2048 * (h // 2 + 1)],
                        )

    nc.compile()
    return nc


def _install_profile_hook():
    """This container's antenv lacks axon_hooks, so run_bass_kernel_spmd's
    trace=True path dies on import. Recreate the module and register the
    ctypes NTFF hook that trn_boot would have installed."""
    import sys as _sys
    import types

    if "antenv.axon_hooks" in _sys.modules:
        return
    import antenv

    mod = types.ModuleType("antenv.axon_hooks")
    mod._hook = None

    def set_axon_ntff_profile_hook(h):
        mod._hook = h

    def get_axon_ntff_profile_hook():
        return mod._hook

    mod.set_axon_ntff_profile_hook = set_axon_ntff_profile_hook
    mod.get_axon_ntff_profile_hook = get_axon_ntff_profile_hook
    _sys.modules["antenv.axon_hooks"] = mod
    antenv.axon_hooks = mod

    from trn_agent_boot.trn_boot import _ntff_profile_via_ctypes

    mod.set_axon_ntff_profile_hook(
        _ntff_profile_via_ctypes("/opt/axon/libaxon_pjrt.so")
    )


_nc = None


def _get_nc():
    global _nc
    if _nc is None:
        _nc = _build()
    return _nc


def _run(x, trace=False, trace_cores=None):
    x = np.asarray(x, dtype=np.float32)
    assert x.shape == (B, C, N), x.shape
    core_ids = list(range(NCORES))
    in_maps = []
    for k in core_ids:
        b, r = divmod(k, 2)
        in_maps.append(
            {
                "xf": np.ascontiguousarray(x[b]),
                "xr": np.ascontiguousarray(x[b][:, r * RB : (r + 1) * RB]),
            }
        )
    if trace:
        _install_profile_hook()
    res = run_bass_kernel_spmd(
        _get_nc(), in_maps, core_ids, trace=trace, trace_cores=trace_cores
    )
    out = np.empty((B, N, N), dtype=np.float32)
    for k in core_ids:
        b, r = divmod(k, 2)
        out[b, r * RB : (r + 1) * RB, :] = res.results[k]["out"]
    return out, res


def kernel(x):
    return _run(x)[0]



# revision 3
# speedup vs baseline: 2.5765x; 2.5765x over previous
"""Cosine-similarity attention map on 8 Trainium2 NeuronCores.

out[b, i, j] = <x[b,:,i], x[b,:,j]> / (||x[b,:,i]|| * ||x[b,:,j]||)
x: [B=4, C=64, N=4096] fp32  ->  out: [B=4, N=4096, N=4096] fp32

The output is symmetric per batch, so each device computes only an upper
triangle (512-col-aligned) in fp16 and the host mirrors + upcasts.  This
cuts HBM writes 4x vs a full fp32 output (the kernel is write-bound).

Sharding: 2 cores per batch.  Core (b, r) owns row tiles t = 2k+r
(k = 0..15, 128 rows each) and computes columns >= 256k for tile k: the
column start is identical for r=0/1, so one SPMD program serves all
cores.  The host pre-normalizes the moving operand (yf16 = x * rsqrt
(sum x^2), fp16), gathers the raw stationary columns (xq16), and ships
per-row scales rsqT [128, 16] fp32; the device fuses the row scale into
the PSUM->SBUF drain (ACT activation-Copy-with-scale / DVE
tensor_scalar_mul), so on-device work is just matmul -> scaled drain ->
fp16 DMA out.
"""

import sys

sys.path.insert(0, "/opt/trn_rl_repo")

import numpy as np

import concourse.bass as bass
import concourse.mybir as mybir
import concourse.tile as tile
from concourse import bacc
from concourse.bass_utils import run_bass_kernel_spmd
from concourse.vector_clock import ScopedClock, VectorClock

B, C, N = 4, 64, 4096
NCORES = 8
KT = 16  # 128-row tiles per core
RB = 128 * KT  # 2048 rows per core
MM_N = 512  # moving free dim per matmul (one PSUM bank of fp32)

F32 = mybir.dt.float32
F16 = mybir.dt.float16


class SplitDrainTileContext(tile.TileContext):
    """Stock TileContext attaches a wait for every pending DMA-queue
    semaphore to a single exit Drain; the walrus build here only allows one
    sync-wait per TPB_CTRL instruction ("Too many sync wait commands").
    Emit one drain per pending logical processor instead."""

    def _drain_and_barrier(self, tick_clock, wait_clock):
        gc = tick_clock.global_clock
        n = len(gc)
        for p in range(n):
            t = gc[p]
            if t <= 0:
                continue
            part = VectorClock([t if q == p else 0 for q in range(n)])
            d = self.nc.sync.drain()
            wait_clock.add_sem_waits(d.ins, ScopedClock({None: part}))

        self.nc.all_engine_barrier()
        assert self.sems is not None
        popped = self.nc._tile_sem_poison_stack.pop()
        assert popped is self._sem_poison
        self.nc.clear_and_free_semaphores(list(self.sems.allocated().values()))
        self.nc.all_engine_barrier()


def _build(use_split_drain=False):
    nc = bacc.Bacc("TRN2", target_bir_lowering=False)
    yf = nc.declare_dram_parameter("yf", [C, N], F16, isOutput=False)
    xq = nc.declare_dram_parameter("xq", [C, RB], F16, isOutput=False)
    rsq = nc.declare_dram_parameter("rsq", [128, KT], F32, isOutput=False)
    out = nc.declare_dram_parameter("out", [RB, N], F16, isOutput=True)

    tc_cls = SplitDrainTileContext if use_split_drain else tile.TileContext
    with tc_cls(nc) as tc:
        with (
            tc.tile_pool(name="persist", bufs=1) as persist,
            tc.tile_pool(name="panels", bufs=3) as panels,
            tc.tile_pool(name="mpsum", bufs=6, space="PSUM") as mpsum,
        ):
            YF = persist.tile([C, N], F16)
            XQ = persist.tile([C, RB], F16)
            RS = persist.tile([128, KT], F32)
            nc.sync.dma_start(out=RS, in_=rsq[:, :])
            nc.sync.dma_start(out=XQ, in_=xq[:, :])
            for c0 in range(0, N, 2048):
                nc.sync.dma_start(
                    out=YF[:, c0 : c0 + 2048], in_=yf[:, c0 : c0 + 2048]
                )

            g = 0  # global drain counter: ACT gets 4/7, DVE 3/7
            for k in range(KT):
                cs = 256 * k
                w = N - cs
                panel = panels.tile([128, N], F16, tag="panel")
                lhsT = XQ[:, 128 * k : 128 * (k + 1)]
                scale = RS[:, k : k + 1]
                offs = list(range(cs, N, MM_N))
                # split the out-DMA so the first half flushes early
                mid = (len(offs) // 2) * MM_N if w >= 2048 else w
                for c0 in offs:
                    cw = min(MM_N, N - c0)
                    ps = mpsum.tile([128, MM_N], F32, tag="ps")
                    nc.tensor.matmul(
                        ps[:, :cw], lhsT=lhsT, rhs=YF[:, c0 : c0 + cw],
                        start=True, stop=True,
                    )
                    dst = panel[:, c0 - cs : c0 - cs + cw]
                    if g % 7 in (0, 2, 4, 6):
                        nc.scalar.activation(
                            dst, ps[:, :cw],
                            mybir.ActivationFunctionType.Copy, scale=scale,
                        )
                    else:
                        nc.vector.tensor_scalar_mul(dst, ps[:, :cw], scale)
                    g += 1
                    if c0 - cs + cw == mid and mid != w:
                        nc.sync.dma_start(
                            out=out[128 * k : 128 * (k + 1), cs : cs + mid],
                            in_=panel[:, :mid],
                        )
                nc.sync.dma_start(
                    out=out[128 * k : 128 * (k + 1), cs + (mid % w) : N],
                    in_=panel[:, (mid % w) : w],
                )

    nc.compile()
    return nc


def _install_profile_hook():
    """This container's antenv lacks axon_hooks, so run_bass_kernel_spmd's
    trace=True path dies on import. Recreate the module and register the
    ctypes NTFF hook that trn_boot would have installed."""
    import sys as _sys
    import types

    if "antenv.axon_hooks" in _sys.modules:
        return
    import antenv

    mod = types.ModuleType("antenv.axon_hooks")
    mod._hook = None

    def set_axon_ntff_profile_hook(h):
        mod._hook = h

    def get_axon_ntff_profile_hook():
        return mod._hook

    mod.set_axon_ntff_profile_hook = set_axon_ntff_profile_hook
    mod.get_axon_ntff_profile_hook = get_axon_ntff_profile_hook
    _sys.modules["antenv.axon_hooks"] = mod
    antenv.axon_hooks = mod

    from trn_agent_boot.trn_boot import _ntff_profile_via_ctypes

    mod.set_axon_ntff_profile_hook(
        _ntff_profile_via_ctypes("/opt/axon/libaxon_pjrt.so")
    )


_nc = None


def _get_nc():
    global _nc
    if _nc is None:
        _nc = _build()
    return _nc


def _run(x, trace=False, trace_cores=None):
    x = np.asarray(x, dtype=np.float32)
    assert x.shape == (B, C, N), x.shape
    rs = 1.0 / np.sqrt(np.einsum("bcn,bcn->bn", x, x))  # [B, N]
    yf16 = (x * rs[:, None, :]).astype(np.float16)  # [B, C, N]
    x16 = x.astype(np.float16)
    core_ids = list(range(NCORES))
    in_maps = []
    for core in core_ids:
        b, r = divmod(core, 2)
        in_maps.append(
            {
                "yf": np.ascontiguousarray(yf16[b]),
                "xq": np.ascontiguousarray(
                    x16[b].reshape(C, KT, 2, 128)[:, :, r, :].reshape(C, RB)
                ),
                "rsq": np.ascontiguousarray(
                    rs[b].reshape(KT, 2, 128)[:, r, :].T
                ).astype(np.float32),
            }
        )
    if trace:
        _install_profile_hook()
    res = run_bass_kernel_spmd(
        _get_nc(), in_maps, core_ids, trace=trace, trace_cores=trace_cores
    )
    out = np.empty((B, N, N), dtype=np.float32)
    for core in core_ids:
        b, r = divmod(core, 2)
        o16 = res.results[core]["out"]  # [2048, 4096] fp16
        for k in range(KT):
            cs = 256 * k
            t = 2 * k + r
            out[b, 128 * t : 128 * t + 128, cs:] = o16[128 * k : 128 * k + 128, cs:]
    for b in range(B):
        ob = out[b]
        for blk in range(1, KT):
            c = 256 * blk
            ob[c : c + 256, :c] = ob[:c, c : c + 256].T
    return out, res


def kernel(x):
    return _run(x)[0]
